# revision 2
# baseline (speedup 1.0000x reference)
"""Trainium2 Bass kernel for nn_Attention_53652731461991 (v2).

Full multi-head attention block (qkv -> per-head LN(q,k) -> softmax(QK^T) -> @V
-> proj) for x [2, 2048, 1024], 16 heads, hd=64. Tensor-parallel over heads:
each of 8 cores computes 2 heads end-to-end plus its row-slice of the output
projection; the host sums the 8 row-split partials and adds b_proj.

v2 design (engine-balanced, bf16-accurate):
  - qkv = x @ w via fp8e4m3 residual pairs (x = xh+xl, w*64 = wh+wl) with
    DoubleRow matmuls: 3 cross terms at 0.5 cyc/row = 0.75x the bf16 cost,
    while matching bf16 accuracy (xl*wl dropped; w scaled x64 so wl clears
    the fp8 subnormal floor).
  - LN stats via one [128,33] selector matmul per tensor-half (mu, mean-sq
    packed at partitions 0/32/64/96), Rsqrt on Act, per-token LN coefficients
    partition-broadcast on the GpSimd/Pool engine, gamma/beta folded in with
    DVE tensor_scalar (4x bf16 mode), apply as two 2x bf16 tensor_tensor ops.
  - S^T[k,q] = k_hat^T @ q_hat in bf16 (fp8 logits fail the 2e-2 gate).
  - exp on Act, with a tunable subset of k-tiles computed on DVE instead via
    a Schraudolph bf16 exponential (one tensor_scalar into an int16 bitcast
    of the bf16 e2 tile) to split the softmax wall across two engines.
  - O[q,hd] accumulated q-on-partitions (F=65 incl. a ones-column denom), so
    the softmax denominator is a per-partition scalar: one DVE reciprocal +
    one tensor_scalar normalizes straight into bf16, then a PE transpose
    returns O^T[c,q] for the row-parallel projection.
  - proj in bf16; y partials DMA'd to DRAM in bf16.
"""

import os
import sys

for _p in ("/opt/trn_rl_repo",):
    if _p not in sys.path and os.path.isdir(_p):
        sys.path.insert(0, _p)

import numpy as np
import ml_dtypes
from contextlib import ExitStack

import concourse.bass as bass
import concourse.bacc as bacc
import concourse.tile as tile
import concourse.mybir as mybir
from concourse.bass_utils import run_bass_kernel_spmd

F32 = mybir.dt.float32
BF16 = mybir.dt.bfloat16
F8 = mybir.dt.float8e4
I16 = mybir.dt.int16
AF = mybir.ActivationFunctionType
OP = mybir.AluOpType
DR = mybir.MatmulPerfMode.DoubleRow

NPF8 = ml_dtypes.float8_e4m3
NPBF = ml_dtypes.bfloat16

P = 128
C = 1024
KO = C // P        # 8 k-subtiles
B = 2
SEQ = 2048
TOKS = B * SEQ     # 4096
TB = 512           # token block
NTB = TOKS // TB   # 8
HD = 64
NQB = SEQ // TB    # 4 q-blocks per batch
NKT = SEQ // P     # 16 k-tiles per batch
NQT = SEQ // P     # 16 q row-tiles per batch (proj)
EPS = 1e-5
NCORES = 8
WS = 64.0          # w_qkv pre-scale so the fp8 residual clears subnormals

# Schraudolph bf16 exp: e2_int16 = floor(A16*s + B16); bitcast -> bf16
SCH_C = -0.0434
A16 = 128.0 / float(np.log(2.0))
B16 = 128.0 * (127.0 + SCH_C) + 0.5
# k-tiles (per q-block) whose exp runs on DVE instead of Act, per segment
SCH_EARLY = (3, 7, 11, 15)
SCH_LATE = (3, 7, 11, 15)
RSQ_MAGIC = 24376


def _emit(tc):
    nc = tc.nc
    xh = nc.dram_tensor("xh", [NTB, P, KO, TB], F8, kind="ExternalInput")
    xl = nc.dram_tensor("xl", [NTB, P, KO, TB], F8, kind="ExternalInput")
    wq = nc.dram_tensor("wq", [P, 2, KO, 384], F8, kind="ExternalInput")
    bq3 = nc.dram_tensor("bq3", [P, 3], F32, kind="ExternalInput")
    selm = nc.dram_tensor("selm", [P, 2], BF16, kind="ExternalInput")
    # grows[r, s, :]: LN coef rows: s=0: g (head-padded), s=1: be
    grows = nc.dram_tensor("grows", [66, 2, P], BF16, kind="ExternalInput")
    gbe = nc.dram_tensor("gbe", [P, 6], F32, kind="ExternalInput")
    wp = nc.dram_tensor("wp", [P, C], BF16, kind="ExternalInput")
    idd = nc.dram_tensor("idd", [P, P], BF16, kind="ExternalInput")
    y = nc.dram_tensor("y", [B, NQT, P, C], BF16, kind="ExternalOutput")

    with ExitStack() as ctx:
        const = ctx.enter_context(tc.tile_pool(name="const", bufs=1))
        resid = ctx.enter_context(tc.tile_pool(name="resid", bufs=1))
        xst = ctx.enter_context(tc.tile_pool(name="xst", bufs=2))
        sqp = ctx.enter_context(tc.tile_pool(name="sqp", bufs=2))
        tst = ctx.enter_context(tc.tile_pool(name="tst", bufs=2))
        coefp = ctx.enter_context(tc.tile_pool(name="coefp", bufs=2))
        ep = ctx.enter_context(tc.tile_pool(name="ep", bufs=24))
        ocp = ctx.enter_context(tc.tile_pool(name="ocp", bufs=2))
        rcp = ctx.enter_context(tc.tile_pool(name="rcp", bufs=2))
        yp = ctx.enter_context(tc.tile_pool(name="yp", bufs=2))
        pm = ctx.enter_context(tc.tile_pool(name="pm", bufs=2, space="PSUM"))
        ps = ctx.enter_context(tc.tile_pool(name="ps", bufs=2, space="PSUM"))
        po = ctx.enter_context(tc.tile_pool(name="po", bufs=1, space="PSUM"))

        # ---- constants ----
        w_sb = const.tile([P, 2, KO, 384], F8)
        nc.sync.dma_start(w_sb[:], wq[:, :, :, :])
        bq_sb = const.tile([P, 3], F32)
        nc.sync.dma_start(bq_sb[:], bq3[:, :])
        sel_sb = const.tile([P, 2], BF16)
        nc.sync.dma_start(sel_sb[:], selm[:, :])
        gr_sb = const.tile([66, 2, P], BF16)
        nc.sync.dma_start(gr_sb[:], grows[:, :, :])
        gbe_sb = const.tile([P, 6], F32)
        nc.sync.dma_start(gbe_sb[:], gbe[:, :])
        wp_sb = const.tile([P, C], BF16)
        nc.sync.dma_start(wp_sb[:], wp[:, :])
        ident = const.tile([P, P], BF16)
        nc.sync.dma_start(ident[:], idd[:, :])

        # ---- residents ----
        qT = resid.tile([P, TOKS], BF16)   # heads 2c (rows 0:64), 2c+1 (64:128)
        kT = resid.tile([P, TOKS], BF16)
        vT = resid.tile([P, TOKS], BF16)
        vtok = resid.tile([P, B * 2, NKT, HD + 1], BF16)  # token-major V | ones
        nc.vector.tensor_copy(vtok[:, :, :, HD:HD + 1],
                              gbe_sb[:, 5:6].to_broadcast((P, B * 2, NKT, 1)))
        OT2 = resid.tile([P, B, SEQ], BF16)  # normalized attention out ^T

        # ---- phase 1 work units (fine-grained, for interleaved emission) ----
        def emit_xcdma(tb):
            xc = xst.tile([P, 2, KO, TB], F8, tag="xc", name=f"xc_{tb}")
            nc.sync.dma_start(xc[:, 0, :, :], xh[tb, :, :, :])
            nc.sync.dma_start(xc[:, 1, :, :], xl[tb, :, :, :])
            return xc

        def emit_qkv_sub(tb, ct, xc, seti, pp):
            # one residual cross-term (4 DR matmuls); seti 2 closes the
            # chain and drains psum through the Act engine
            ts = slice(tb * TB, (tb + 1) * TB)
            dest = (qT, kT, vT)[ct]
            a, b = ((0, 0), (1, 0), (0, 1))[seti]  # xh*wh, xl*wh, xh*wl
            for kp in range(KO // 2):
                nc.tensor.matmul(
                    pp[:],
                    lhsT=w_sb[:, b, 2 * kp:2 * kp + 2,
                              ct * P:(ct + 1) * P],
                    rhs=xc[:, a, 2 * kp:2 * kp + 2, :],
                    start=(seti, kp) == (0, 0),
                    stop=(seti, kp) == (2, KO // 2 - 1),
                    perf_mode=DR,
                )
            if seti == 2:
                nc.scalar.activation(dest[:, ts], pp[:], AF.Identity,
                                     bias=bq_sb[:, ct:ct + 1], scale=1.0 / WS)

        def emit_stats_mu(tb):
            ts = slice(tb * TB, (tb + 1) * TB)
            st_mu = ps.tile([P, 2 * TB], F32, tag="s", name=f"stmu_{tb}")
            nc.tensor.matmul(st_mu[0:2, 0:TB], lhsT=sel_sb[:], rhs=qT[:, ts],
                             start=True, stop=True)
            nc.tensor.matmul(st_mu[64:66, 0:TB], lhsT=sel_sb[:], rhs=kT[:, ts],
                             start=True, stop=True)
            return st_mu

        def emit_stats_ms(tb, pfx=False):
            eng = nc.gpsimd if pfx else nc.vector
            ts = slice(tb * TB, (tb + 1) * TB)
            sqq = sqp.tile([P, TB], BF16, tag="sq", name=f"sqq_{tb}")
            eng.tensor_tensor(sqq[:], qT[:, ts], qT[:, ts], OP.mult)
            sqk = sqp.tile([P, TB], BF16, tag="sq", name=f"sqk_{tb}")
            eng.tensor_tensor(sqk[:], kT[:, ts], kT[:, ts], OP.mult)
            st_ms = ps.tile([P, 2 * TB], F32, tag="s", name=f"stms_{tb}")
            nc.tensor.matmul(st_ms[0:2, 0:TB], lhsT=sel_sb[:], rhs=sqq[:],
                             start=True, stop=True)
            nc.tensor.matmul(st_ms[64:66, 0:TB], lhsT=sel_sb[:], rhs=sqk[:],
                             start=True, stop=True)
            return st_ms

        def emit_statpipe(tb, st_mu, st_ms, pfx=False):
            t_sb = tst.tile([P, 2, TB], BF16, tag="t", name=f"tsb_{tb}")
            nc.scalar.activation(t_sb[:, 0, :], st_mu[:, 0:TB], AF.Identity,
                                 bias=0.0, scale=1.0)
            nc.scalar.activation(t_sb[:, 1, :], st_ms[:, 0:TB], AF.Identity,
                                 bias=0.0, scale=1.0)
            tm = tst.tile([P, TB], BF16, tag="tm", name=f"tm_{tb}")
            r0 = tst.tile([P, TB], BF16, tag="r0", name=f"r0_{tb}")
            nc.vector.tensor_tensor(tm[:], t_sb[:, 0, :], t_sb[:, 0, :],
                                    OP.mult)                      # mu^2
            nc.vector.tensor_tensor(t_sb[:, 1, :], t_sb[:, 1, :], tm[:],
                                    OP.subtract)                  # var
            # rs = rsqrt(var) via bf16 bit-hack + one Newton step (all DVE,
            # keeps the Act engine exp-table resident)
            nc.vector.tensor_scalar(r0.bitcast(I16)[:], t_sb.bitcast(I16)[:, 1, :],
                                    -0.5, float(RSQ_MAGIC), OP.mult, OP.add)
            nc.vector.tensor_tensor(tm[:], r0[:], r0[:], OP.mult)   # r0^2
            nc.vector.tensor_tensor(tm[:], tm[:], t_sb[:, 1, :], OP.mult)
            nc.vector.tensor_scalar(tm[:], tm[:], -0.5, 1.5, OP.mult, OP.add)
            nc.vector.tensor_tensor(t_sb[:, 1, :], r0[:], tm[:], OP.mult)  # rs
            nc.vector.scalar_tensor_tensor(t_sb[:, 0, :], t_sb[:, 0, :],
                                           -1.0, t_sb[:, 1, :],
                                           OP.mult, OP.mult)      # nb = -mu*rs
            return t_sb

        def emit_coef_apply(tb, tsr, t_sb, pfx=False):
            # per-token LN coefficients via PE ones-matmul broadcast
            # (g folded into the stationary rows); be added per-partition.
            ts = slice(tb * TB, (tb + 1) * TB)
            r0, gc = (0, 0) if tsr == 0 else (64, 2)
            c_rs = ps.tile([P, 2 * TB], F32, tag="s", name=f"crs_{tb}_{tsr}")
            c_nb = ps.tile([P, 2 * TB], F32, tag="s", name=f"cnb_{tb}_{tsr}")
            nc.tensor.matmul(c_rs[:, 0:TB],
                             lhsT=gr_sb[r0:r0 + 2, 0, :],
                             rhs=t_sb[r0:r0 + 2, 1, :],
                             start=True, stop=True)
            nc.tensor.matmul(c_nb[:, 0:TB],
                             lhsT=gr_sb[r0:r0 + 2, 0, :],
                             rhs=t_sb[r0:r0 + 2, 0, :],
                             start=True, stop=True)
            tgt = (qT if tsr == 0 else kT)[:, ts]
            nc.vector.tensor_tensor(tgt, tgt, c_rs[:, 0:TB], OP.mult)
            nc.vector.scalar_tensor_tensor(tgt, tgt,
                                           gbe_sb[:, gc + 1:gc + 2],
                                           c_nb[:, 0:TB],
                                           OP.add, OP.add)

        def emit_vt(tb, half, pfx=False):
            # V transposes for half of this block's tokens
            b2 = tb // (NTB // B)
            for h in range(2):
                hb = HD * h
                for kt in range((tb % 4) * 4 + 2 * half,
                                (tb % 4) * 4 + 2 * half + 2):
                    kts = slice(b2 * SEQ + kt * P, b2 * SEQ + (kt + 1) * P)
                    pt = pm.tile([P, HD], BF16, tag="m",
                                 name=f"vt_{tb}_{h}_{kt}")
                    nc.tensor.transpose(pt[:], vT[hb:hb + HD, kts],
                                        ident[hb:hb + HD, hb:hb + HD])
                    nc.vector.tensor_copy(vtok[:, b2 * 2 + h, kt, 0:HD],
                                          pt[:])

        def phase1_units(tb, pfx=False):
            state = {}
            units = []

            def u_dma():
                state["xc"] = emit_xcdma(tb)

            units.append(u_dma)
            for ct in range(3):
                def u_alloc(ct=ct):
                    state[f"pp{ct}"] = pm.tile([P, TB], F32, tag="m",
                                               name=f"qkv_{tb}_{ct}")
                    emit_qkv_sub(tb, ct, state["xc"], 0, state[f"pp{ct}"])
                units.append(u_alloc)
                for seti in (1, 2):
                    units.append(lambda ct=ct, seti=seti: emit_qkv_sub(
                        tb, ct, state["xc"], seti, state[f"pp{ct}"]))

            def u_stmu():
                state["mu"] = emit_stats_mu(tb)

            def u_stms():
                state["ms"] = emit_stats_ms(tb, pfx)

            def u_pipe():
                state["t"] = emit_statpipe(tb, state["mu"], state["ms"], pfx)

            units += [u_stmu, u_stms, u_pipe,
                      lambda: emit_coef_apply(tb, 0, state["t"], pfx),
                      lambda: emit_coef_apply(tb, 1, state["t"], pfx),
                      lambda: emit_vt(tb, 0, pfx),
                      lambda: emit_vt(tb, 1, pfx)]
            return units

        # ---- phase 3: projection partial -> y (bf16) ----
        proj_state = {}

        def emit_proj_half(b2, qt, half):
            if half == 0:
                proj_state[(b2, qt)] = yp.tile([P, C], BF16, tag="y",
                                               name=f"yt_{b2}_{qt}")
            yt = proj_state[(b2, qt)]
            pp = pm.tile([P, TB], F32, tag="m", name=f"pp_{b2}_{qt}_{half}")
            nc.tensor.matmul(
                pp[:],
                lhsT=OT2[:, b2, qt * P:(qt + 1) * P],
                rhs=wp_sb[:, half * TB:(half + 1) * TB],
                start=True, stop=True)
            if half == 0:
                nc.scalar.activation(yt[:, 0:TB], pp[:], AF.Identity,
                                     bias=0.0, scale=1.0)
            else:
                nc.vector.tensor_copy(yt[:, TB:C], pp[:])
                nc.sync.dma_start(y[b2, qt, :, :], yt[:])

        # ---- phase 2: attention, software-pipelined at the q-block level.
        # emit_seg runs the S/exp loop for one (batch, q-block) with filler
        # units woven after every k-tile; the segment's O-chains, normalize
        # and transpose work are RETURNED as units and run as fillers inside
        # the NEXT segment, so PE never stalls on the softmax barrier. ----
        def emit_seg(b2, qb, fillers=(), sch=SCH_EARLY, gates=None):
            fillers = list(fillers)
            gates = gates or {}
            qs = slice(b2 * SEQ + qb * TB, b2 * SEQ + (qb + 1) * TB)
            o_ps = [po.tile([P, NQB, HD + 1], F32, tag=f"o{h}",
                            name=f"o_{b2}_{qb}_{h}") for h in range(2)]
            e2s = []
            for kt in range(NKT):
                for u in gates.get(kt, ()):
                    u()
                kts = slice(b2 * SEQ + kt * P, b2 * SEQ + (kt + 1) * P)
                s2 = ps.tile([P, 2, TB], F32, tag="s", name=f"s2_{b2}_{qb}_{kt}")
                for h in range(2):
                    hb = HD * h
                    nc.tensor.matmul(s2[:, h, :],
                                     lhsT=kT[hb:hb + HD, kts],
                                     rhs=qT[hb:hb + HD, qs],
                                     start=True, stop=True)
                e2 = ep.tile([P, 2, TB], BF16, tag="e",
                             name=f"e2_{b2}_{qb}_{kt}")
                if kt in sch:
                    nc.vector.tensor_scalar(e2.bitcast(I16)[:, :, :], s2[:],
                                            A16, B16, OP.mult, OP.add)
                else:
                    nc.scalar.activation(e2[:], s2[:], AF.Exp,
                                         bias=0.0, scale=1.0)
                e2s.append(e2)
                for _ in range(2):
                    if fillers:
                        fillers.pop(0)()
            for f in fillers:
                f()

            def u_chain(qx, k0):
                # one accumulation group open per PSUM bank at a time; the
                # 4-ktile sub-chains stay in queue order so each (h, qx)
                # group runs without same-bank interleaving
                for kt in range(k0, k0 + 4):
                    for h in range(2):
                        nc.tensor.matmul(
                            o_ps[h][:, qx, :],
                            lhsT=e2s[kt][:, h, qx * P:(qx + 1) * P],
                            rhs=vtok[:, b2 * 2 + h, kt, :],
                            start=(kt == 0), stop=(kt == NKT - 1))

            oc = ocp.tile([P, NQB, 2, HD], BF16, tag="oc",
                          name=f"oc_{b2}_{qb}")

            def u_norm(h):
                # normalize by the ones-column denom (per-partition scalar)
                rc = rcp.tile([P, NQB], F32, tag="rc",
                              name=f"rc_{b2}_{qb}_{h}")
                nc.vector.reciprocal(rc[:], o_ps[h][:, :, HD:HD + 1])
                for qx in range(NQB):
                    nc.vector.tensor_scalar(oc[:, qx, h, :],
                                            o_ps[h][:, qx, 0:HD],
                                            rc[:, qx:qx + 1], None,
                                            OP.mult)

            def u_tr(qx):
                tr = pm.tile([P, P], BF16, tag="m",
                             name=f"tr_{b2}_{qb}_{qx}")
                nc.tensor.transpose(tr[:], oc[:, qx, :, :], ident[:, :])
                nc.vector.tensor_copy(
                    OT2[:, b2, qb * TB + qx * P: qb * TB + (qx + 1) * P],
                    tr[:])

            return ([lambda qx=qx, k0=k0: u_chain(qx, k0)
                     for qx in range(NQB) for k0 in range(0, NKT, 4)]
                    + [lambda h=h: u_norm(h) for h in range(2)]
                    + [lambda qx=qx: u_tr(qx) for qx in range(NQB)])

        # ---- interleaved, segment-pipelined emission ----
        def weave(a, b):
            out = []
            ia = ib = 0
            while ia < len(a) or ib < len(b):
                if ia < len(a):
                    out.append(a[ia]); ia += 1
                if ib < len(b):
                    out.append(b[ib]); ib += 1
            return out

        for tb in range(4):
            for u in phase1_units(tb, pfx=True):
                u()
        segs = [(0, qb) for qb in range(NQB)] + [(1, qb) for qb in range(NQB)]
        proj_units = (
            [lambda b2=b2, qt=qt, hf=hf: emit_proj_half(b2, qt, hf)
             for b2 in range(B) for qt in range(NQT) for hf in range(2)])
        # proj(b2, qt-group g) is eligible once tail(seg 4*b2+g) ran; projs
        # stay strictly after the tail units in each filler list
        per_seg_proj = {4: 10, 5: 16, 6: 16, 7: 12}
        tail = []
        for i, (b2, qb) in enumerate(segs):
            if i < 4:
                fillers = weave(tail, phase1_units(4 + i))
            else:
                n = min(per_seg_proj.get(i, 0), len(proj_units))
                fillers = tail + proj_units[:n]
                del proj_units[:n]
            tail = emit_seg(b2, qb, fillers,
                            sch=SCH_EARLY if i < 4 else SCH_LATE)
        for f in tail + proj_units:
            f()

        if os.environ.get("KV2_DEBUG2"):
            dts = nc.dram_tensor("dbg_tsb", [P, 2, TB], BF16,
                                 kind="ExternalOutput")
            dcq = nc.dram_tensor("dbg_coefq", [P, 2, TB], BF16,
                                 kind="ExternalOutput")
            dbg_c = const.tile([P, 2, TB], BF16, name="dbg_c")
            nc.vector.tensor_copy(dbg_c[:], dbg_tiles[1][:])
            nc.sync.dma_start(dts[:, :, :], dbg_tiles[0][:])
            nc.sync.dma_start(dcq[:, :, :], dbg_c[:])
        if os.environ.get("KV2_DEBUG"):
            dq = nc.dram_tensor("dbg_q", [P, TOKS], BF16, kind="ExternalOutput")
            dk = nc.dram_tensor("dbg_k", [P, TOKS], BF16, kind="ExternalOutput")
            dv = nc.dram_tensor("dbg_v", [P, TOKS], BF16, kind="ExternalOutput")
            dvt = nc.dram_tensor("dbg_vtok", [P, B * 2, NKT, HD + 1], BF16,
                                 kind="ExternalOutput")
            dot = nc.dram_tensor("dbg_ot2", [P, B, SEQ], BF16,
                                 kind="ExternalOutput")
            nc.sync.dma_start(dq[:, :], qT[:])
            nc.sync.dma_start(dk[:, :], kT[:])
            nc.sync.dma_start(dv[:, :], vT[:])
            nc.sync.dma_start(dvt[:, :, :, :], vtok[:])
            nc.sync.dma_start(dot[:, :, :], OT2[:])


_NC_CACHE = None


def build_nc():
    global _NC_CACHE
    if _NC_CACHE is None:
        nc = bacc.Bacc("TRN2", target_bir_lowering=False, debug=False)
        with tile.TileContext(nc) as tc:
            _emit(tc)
        nc.compile()
        _NC_CACHE = nc
    return _NC_CACHE


def make_in_maps(x, w_qkv, b_qkv, g_q, be_q, g_k, be_k, w_proj):
    x2 = np.ascontiguousarray(np.asarray(x, np.float32).reshape(TOKS, C))
    w_qkv = np.asarray(w_qkv, np.float32)
    b_qkv = np.asarray(b_qkv, np.float32)
    g_q = np.asarray(g_q, np.float32)
    be_q = np.asarray(be_q, np.float32)
    g_k = np.asarray(g_k, np.float32)
    be_k = np.asarray(be_k, np.float32)
    w_proj = np.asarray(w_proj, np.float32)

    # x fp8 residual pair, laid out [tb, p, ko, t]
    xh_f = x2.astype(NPF8)
    xl_f = (x2 - xh_f.astype(np.float32)).astype(NPF8)

    def to_xT(a):
        return np.ascontiguousarray(
            a.T.reshape(KO, P, NTB, TB).transpose(2, 1, 0, 3))

    xh_h = to_xT(xh_f)
    xl_h = to_xT(xl_f)

    sel_h = np.zeros((P, 2), NPBF)
    sel_h[0:HD, 0] = np.float32(1.0 / HD)
    sel_h[HD:P, 1] = np.float32(1.0 / HD)
    grows_h = np.zeros((66, 2, P), NPBF)
    for r0, gv in ((0, g_q / 8.0), (64, g_k)):
        grows_h[r0, 0, 0:HD] = gv
        grows_h[r0 + 1, 0, HD:P] = gv
    gbe_h = np.stack([np.tile(g_q, 2) / 8.0, np.tile(be_q, 2) / 8.0,
                      np.tile(g_k, 2), np.tile(be_k, 2),
                      np.full(P, EPS, np.float32),
                      np.ones(P, np.float32)], axis=1)
    gbe_h = np.ascontiguousarray(gbe_h.astype(np.float32))
    idd_h = np.eye(P, dtype=NPBF)

    in_maps = []
    for c in range(NCORES):
        cs = slice(P * c, P * (c + 1))
        wcat = np.concatenate(
            [w_qkv[:, 0:C][:, cs], w_qkv[:, C:2 * C][:, cs],
             w_qkv[:, 2 * C:3 * C][:, cs]], axis=1) * np.float32(WS)
        wh_f = wcat.astype(NPF8)
        wl_f = (wcat - wh_f.astype(np.float32)).astype(NPF8)
        # [p, hl, ko, col]
        w_h = np.ascontiguousarray(np.stack(
            [wh_f.reshape(KO, P, 384).transpose(1, 0, 2),
             wl_f.reshape(KO, P, 384).transpose(1, 0, 2)], axis=1))
        bcat = np.concatenate(
            [b_qkv[0:C][cs], b_qkv[C:2 * C][cs], b_qkv[2 * C:3 * C][cs]])
        b_h = np.ascontiguousarray(bcat.reshape(3, P).T.astype(np.float32))
        wp_h = np.ascontiguousarray(w_proj[cs, :].astype(NPBF))
        in_maps.append({
            "xh": xh_h, "xl": xl_h, "wq": w_h, "bq3": b_h,
            "selm": sel_h, "grows": grows_h, "gbe": gbe_h,
            "wp": wp_h, "idd": idd_h,
        })
    return in_maps


def kernel(x, w_qkv, b_qkv, g_q, be_q, g_k, be_k, w_proj, b_proj, **run_kwargs):
    in_maps = make_in_maps(x, w_qkv, b_qkv, g_q, be_q, g_k, be_k, w_proj)
    nc = build_nc()
    res = run_bass_kernel_spmd(nc, in_maps, list(range(NCORES)), **run_kwargs)
    acc = np.zeros((TOKS, C), np.float32)
    for r in res.results:
        yv = r["y"]
        if yv.dtype != NPBF:
            yv = yv.view(NPBF)
        acc += yv.reshape(TOKS, C).astype(np.float32)
    out = acc + np.asarray(b_proj, np.float32)
    out = out.astype(np.float32).reshape(B, SEQ, C)
    kernel.last_result = res
    return out


# revision 3
# speedup vs baseline: 1.0070x; 1.0070x over previous
"""Trainium2 Bass kernel for nn_Attention_53652731461991 (v2).

Full multi-head attention block (qkv -> per-head LN(q,k) -> softmax(QK^T) -> @V
-> proj) for x [2, 2048, 1024], 16 heads, hd=64. Tensor-parallel over heads:
each of 8 cores computes 2 heads end-to-end plus its row-slice of the output
projection; the host sums the 8 row-split partials and adds b_proj.

v2 design (engine-balanced, bf16-accurate):
  - qkv = x @ w via fp8e4m3 residual pairs (x = xh+xl, w*64 = wh+wl) with
    DoubleRow matmuls: 3 cross terms at 0.5 cyc/row = 0.75x the bf16 cost,
    while matching bf16 accuracy (xl*wl dropped; w scaled x64 so wl clears
    the fp8 subnormal floor).
  - LN stats via one [128,33] selector matmul per tensor-half (mu, mean-sq
    packed at partitions 0/32/64/96), Rsqrt on Act, per-token LN coefficients
    partition-broadcast on the GpSimd/Pool engine, gamma/beta folded in with
    DVE tensor_scalar (4x bf16 mode), apply as two 2x bf16 tensor_tensor ops.
  - S^T[k,q] = k_hat^T @ q_hat in bf16 (fp8 logits fail the 2e-2 gate).
  - exp on Act, with a tunable subset of k-tiles computed on DVE instead via
    a Schraudolph bf16 exponential (one tensor_scalar into an int16 bitcast
    of the bf16 e2 tile) to split the softmax wall across two engines.
  - O[q,hd] accumulated q-on-partitions (F=65 incl. a ones-column denom), so
    the softmax denominator is a per-partition scalar: one DVE reciprocal +
    one tensor_scalar normalizes straight into bf16, then a PE transpose
    returns O^T[c,q] for the row-parallel projection.
  - proj in bf16; y partials DMA'd to DRAM in bf16.
"""

import os
import sys

for _p in ("/opt/trn_rl_repo",):
    if _p not in sys.path and os.path.isdir(_p):
        sys.path.insert(0, _p)

import numpy as np
import ml_dtypes
from contextlib import ExitStack

import concourse.bass as bass
import concourse.bacc as bacc
import concourse.tile as tile
import concourse.mybir as mybir
from concourse.bass_utils import run_bass_kernel_spmd

F32 = mybir.dt.float32
BF16 = mybir.dt.bfloat16
F8 = mybir.dt.float8e4
I16 = mybir.dt.int16
AF = mybir.ActivationFunctionType
OP = mybir.AluOpType
DR = mybir.MatmulPerfMode.DoubleRow

NPF8 = ml_dtypes.float8_e4m3
NPBF = ml_dtypes.bfloat16

P = 128
C = 1024
KO = C // P        # 8 k-subtiles
B = 2
SEQ = 2048
TOKS = B * SEQ     # 4096
TB = 512           # token block
NTB = TOKS // TB   # 8
HD = 64
NQB = SEQ // TB    # 4 q-blocks per batch
NKT = SEQ // P     # 16 k-tiles per batch
NQT = SEQ // P     # 16 q row-tiles per batch (proj)
EPS = 1e-5
NCORES = 8
WS = 64.0          # w_qkv pre-scale so the fp8 residual clears subnormals

# Schraudolph bf16 exp: e2_int16 = floor(A16*s + B16); bitcast -> bf16
SCH_C = -0.0434
A16 = 128.0 / float(np.log(2.0))
B16 = 128.0 * (127.0 + SCH_C) + 0.5
# k-tiles (per q-block) whose exp runs on DVE instead of Act, per segment
SCH_EARLY = (3, 7, 11, 15)
SCH_LATE = (2, 5, 9, 12, 15)
RSQ_MAGIC = 24376


def _emit(tc):
    nc = tc.nc
    xh = nc.dram_tensor("xh", [NTB, P, KO, TB], F8, kind="ExternalInput")
    xl = nc.dram_tensor("xl", [NTB, P, KO, TB], F8, kind="ExternalInput")
    wq = nc.dram_tensor("wq", [P, 2, KO, 384], F8, kind="ExternalInput")
    bq3 = nc.dram_tensor("bq3", [P, 3], F32, kind="ExternalInput")
    selm = nc.dram_tensor("selm", [P, 2], BF16, kind="ExternalInput")
    # grows[r, s, :]: LN coef rows: s=0: g (head-padded), s=1: be
    grows = nc.dram_tensor("grows", [66, 2, P], BF16, kind="ExternalInput")
    gbe = nc.dram_tensor("gbe", [P, 6], F32, kind="ExternalInput")
    wp = nc.dram_tensor("wp", [P, C], BF16, kind="ExternalInput")
    idd = nc.dram_tensor("idd", [P, P], BF16, kind="ExternalInput")
    y = nc.dram_tensor("y", [B, NQT, P, C], BF16, kind="ExternalOutput")

    with ExitStack() as ctx:
        const = ctx.enter_context(tc.tile_pool(name="const", bufs=1))
        resid = ctx.enter_context(tc.tile_pool(name="resid", bufs=1))
        xst = ctx.enter_context(tc.tile_pool(name="xst", bufs=2))
        sqp = ctx.enter_context(tc.tile_pool(name="sqp", bufs=2))
        tst = ctx.enter_context(tc.tile_pool(name="tst", bufs=2))
        coefp = ctx.enter_context(tc.tile_pool(name="coefp", bufs=2))
        ep = ctx.enter_context(tc.tile_pool(name="ep", bufs=28))
        ocp = ctx.enter_context(tc.tile_pool(name="ocp", bufs=2))
        rcp = ctx.enter_context(tc.tile_pool(name="rcp", bufs=2))
        yp = ctx.enter_context(tc.tile_pool(name="yp", bufs=2))
        pm = ctx.enter_context(tc.tile_pool(name="pm", bufs=2, space="PSUM"))
        ps = ctx.enter_context(tc.tile_pool(name="ps", bufs=2, space="PSUM"))
        po = ctx.enter_context(tc.tile_pool(name="po", bufs=1, space="PSUM"))

        # ---- constants ----
        w_sb = const.tile([P, 2, KO, 384], F8)
        nc.sync.dma_start(w_sb[:], wq[:, :, :, :])
        bq_sb = const.tile([P, 3], F32)
        nc.sync.dma_start(bq_sb[:], bq3[:, :])
        sel_sb = const.tile([P, 2], BF16)
        nc.sync.dma_start(sel_sb[:], selm[:, :])
        gr_sb = const.tile([66, 2, P], BF16)
        nc.sync.dma_start(gr_sb[:], grows[:, :, :])
        gbe_sb = const.tile([P, 6], F32)
        nc.sync.dma_start(gbe_sb[:], gbe[:, :])
        wp_sb = const.tile([P, C], BF16)
        nc.sync.dma_start(wp_sb[:], wp[:, :])
        ident = const.tile([P, P], BF16)
        nc.sync.dma_start(ident[:], idd[:, :])

        # ---- residents ----
        qT = resid.tile([P, TOKS], BF16)   # heads 2c (rows 0:64), 2c+1 (64:128)
        kT = resid.tile([P, TOKS], BF16)
        vT = resid.tile([P, TOKS], BF16)
        vtok = resid.tile([P, B * 2, NKT, HD + 1], BF16)  # token-major V | ones
        nc.vector.tensor_copy(vtok[:, :, :, HD:HD + 1],
                              gbe_sb[:, 5:6].to_broadcast((P, B * 2, NKT, 1)))
        OT2 = resid.tile([P, B, SEQ], BF16)  # normalized attention out ^T

        # ---- phase 1 work units (fine-grained, for interleaved emission) ----
        def emit_xcdma(tb):
            xc = xst.tile([P, 2, KO, TB], F8, tag="xc", name=f"xc_{tb}")
            nc.sync.dma_start(xc[:, 0, :, :], xh[tb, :, :, :])
            nc.sync.dma_start(xc[:, 1, :, :], xl[tb, :, :, :])
            return xc

        def emit_qkv_sub(tb, ct, xc, seti, pp):
            # one residual cross-term (4 DR matmuls); seti 2 closes the
            # chain and drains psum through the Act engine
            ts = slice(tb * TB, (tb + 1) * TB)
            dest = (qT, kT, vT)[ct]
            a, b = ((0, 0), (1, 0), (0, 1))[seti]  # xh*wh, xl*wh, xh*wl
            for kp in range(KO // 2):
                nc.tensor.matmul(
                    pp[:],
                    lhsT=w_sb[:, b, 2 * kp:2 * kp + 2,
                              ct * P:(ct + 1) * P],
                    rhs=xc[:, a, 2 * kp:2 * kp + 2, :],
                    start=(seti, kp) == (0, 0),
                    stop=(seti, kp) == (2, KO // 2 - 1),
                    perf_mode=DR,
                )
            if seti == 2:
                nc.scalar.activation(dest[:, ts], pp[:], AF.Identity,
                                     bias=bq_sb[:, ct:ct + 1], scale=1.0 / WS)

        def emit_stats_mu(tb):
            ts = slice(tb * TB, (tb + 1) * TB)
            st_mu = ps.tile([P, 2 * TB], F32, tag="s", name=f"stmu_{tb}")
            nc.tensor.matmul(st_mu[0:2, 0:TB], lhsT=sel_sb[:], rhs=qT[:, ts],
                             start=True, stop=True)
            nc.tensor.matmul(st_mu[64:66, 0:TB], lhsT=sel_sb[:], rhs=kT[:, ts],
                             start=True, stop=True)
            return st_mu

        def emit_stats_ms(tb, pfx=False):
            eng = nc.gpsimd if pfx else nc.vector
            ts = slice(tb * TB, (tb + 1) * TB)
            sqq = sqp.tile([P, TB], BF16, tag="sq", name=f"sqq_{tb}")
            eng.tensor_tensor(sqq[:], qT[:, ts], qT[:, ts], OP.mult)
            sqk = sqp.tile([P, TB], BF16, tag="sq", name=f"sqk_{tb}")
            eng.tensor_tensor(sqk[:], kT[:, ts], kT[:, ts], OP.mult)
            st_ms = ps.tile([P, 2 * TB], F32, tag="s", name=f"stms_{tb}")
            nc.tensor.matmul(st_ms[0:2, 0:TB], lhsT=sel_sb[:], rhs=sqq[:],
                             start=True, stop=True)
            nc.tensor.matmul(st_ms[64:66, 0:TB], lhsT=sel_sb[:], rhs=sqk[:],
                             start=True, stop=True)
            return st_ms

        def emit_statpipe(tb, st_mu, st_ms, pfx=False):
            t_sb = tst.tile([P, 2, TB], BF16, tag="t", name=f"tsb_{tb}")
            nc.scalar.activation(t_sb[:, 0, :], st_mu[:, 0:TB], AF.Identity,
                                 bias=0.0, scale=1.0)
            nc.scalar.activation(t_sb[:, 1, :], st_ms[:, 0:TB], AF.Identity,
                                 bias=0.0, scale=1.0)
            tm = tst.tile([P, TB], BF16, tag="tm", name=f"tm_{tb}")
            r0 = tst.tile([P, TB], BF16, tag="r0", name=f"r0_{tb}")
            nc.vector.tensor_tensor(tm[:], t_sb[:, 0, :], t_sb[:, 0, :],
                                    OP.mult)                      # mu^2
            nc.vector.tensor_tensor(t_sb[:, 1, :], t_sb[:, 1, :], tm[:],
                                    OP.subtract)                  # var
            # rs = rsqrt(var) via bf16 bit-hack + one Newton step (all DVE,
            # keeps the Act engine exp-table resident)
            nc.vector.tensor_scalar(r0.bitcast(I16)[:], t_sb.bitcast(I16)[:, 1, :],
                                    -0.5, float(RSQ_MAGIC), OP.mult, OP.add)
            nc.vector.tensor_tensor(tm[:], r0[:], r0[:], OP.mult)   # r0^2
            nc.vector.tensor_tensor(tm[:], tm[:], t_sb[:, 1, :], OP.mult)
            nc.vector.tensor_scalar(tm[:], tm[:], -0.5, 1.5, OP.mult, OP.add)
            nc.vector.tensor_tensor(t_sb[:, 1, :], r0[:], tm[:], OP.mult)  # rs
            nc.vector.scalar_tensor_tensor(t_sb[:, 0, :], t_sb[:, 0, :],
                                           -1.0, t_sb[:, 1, :],
                                           OP.mult, OP.mult)      # nb = -mu*rs
            return t_sb

        def emit_coef_apply(tb, tsr, t_sb, pfx=False):
            # per-token LN coefficients via PE ones-matmul broadcast
            # (g folded into the stationary rows); be added per-partition.
            ts = slice(tb * TB, (tb + 1) * TB)
            r0, gc = (0, 0) if tsr == 0 else (64, 2)
            c_rs = ps.tile([P, 2 * TB], F32, tag="s", name=f"crs_{tb}_{tsr}")
            c_nb = ps.tile([P, 2 * TB], F32, tag="s", name=f"cnb_{tb}_{tsr}")
            nc.tensor.matmul(c_rs[:, 0:TB],
                             lhsT=gr_sb[r0:r0 + 2, 0, :],
                             rhs=t_sb[r0:r0 + 2, 1, :],
                             start=True, stop=True)
            nc.tensor.matmul(c_nb[:, 0:TB],
                             lhsT=gr_sb[r0:r0 + 2, 0, :],
                             rhs=t_sb[r0:r0 + 2, 0, :],
                             start=True, stop=True)
            tgt = (qT if tsr == 0 else kT)[:, ts]
            nc.vector.tensor_tensor(tgt, tgt, c_rs[:, 0:TB], OP.mult)
            nc.vector.scalar_tensor_tensor(tgt, tgt,
                                           gbe_sb[:, gc + 1:gc + 2],
                                           c_nb[:, 0:TB],
                                           OP.add, OP.add)

        def emit_vt(tb, half, pfx=False):
            # V transposes for half of this block's tokens
            b2 = tb // (NTB // B)
            for h in range(2):
                hb = HD * h
                for kt in range((tb % 4) * 4 + 2 * half,
                                (tb % 4) * 4 + 2 * half + 2):
                    kts = slice(b2 * SEQ + kt * P, b2 * SEQ + (kt + 1) * P)
                    pt = pm.tile([P, HD], BF16, tag="m",
                                 name=f"vt_{tb}_{h}_{kt}")
                    nc.tensor.transpose(pt[:], vT[hb:hb + HD, kts],
                                        ident[hb:hb + HD, hb:hb + HD])
                    nc.vector.tensor_copy(vtok[:, b2 * 2 + h, kt, 0:HD],
                                          pt[:])

        def phase1_units(tb, pfx=False):
            state = {}
            units = []

            def u_dma():
                state["xc"] = emit_xcdma(tb)

            units.append(u_dma)
            for ct in range(3):
                def u_alloc(ct=ct):
                    state[f"pp{ct}"] = pm.tile([P, TB], F32, tag="m",
                                               name=f"qkv_{tb}_{ct}")
                    emit_qkv_sub(tb, ct, state["xc"], 0, state[f"pp{ct}"])
                units.append(u_alloc)
                for seti in (1, 2):
                    units.append(lambda ct=ct, seti=seti: emit_qkv_sub(
                        tb, ct, state["xc"], seti, state[f"pp{ct}"]))

            def u_stmu():
                state["mu"] = emit_stats_mu(tb)

            def u_stms():
                state["ms"] = emit_stats_ms(tb, pfx)

            def u_pipe():
                state["t"] = emit_statpipe(tb, state["mu"], state["ms"], pfx)

            units += [u_stmu, u_stms, u_pipe,
                      lambda: emit_coef_apply(tb, 0, state["t"], pfx),
                      lambda: emit_coef_apply(tb, 1, state["t"], pfx),
                      lambda: emit_vt(tb, 0, pfx),
                      lambda: emit_vt(tb, 1, pfx)]
            return units

        # ---- phase 3: projection partial -> y (bf16) ----
        proj_state = {}

        def emit_proj_half(b2, qt, half):
            if half == 0:
                proj_state[(b2, qt)] = yp.tile([P, C], BF16, tag="y",
                                               name=f"yt_{b2}_{qt}")
            yt = proj_state[(b2, qt)]
            pp = pm.tile([P, TB], F32, tag="m", name=f"pp_{b2}_{qt}_{half}")
            nc.tensor.matmul(
                pp[:],
                lhsT=OT2[:, b2, qt * P:(qt + 1) * P],
                rhs=wp_sb[:, half * TB:(half + 1) * TB],
                start=True, stop=True)
            if half == 0:
                nc.scalar.activation(yt[:, 0:TB], pp[:], AF.Identity,
                                     bias=0.0, scale=1.0)
            else:
                nc.vector.tensor_copy(yt[:, TB:C], pp[:])
                nc.sync.dma_start(y[b2, qt, :, :], yt[:])

        # ---- phase 2: attention, software-pipelined at the q-block level.
        # emit_seg runs the S/exp loop for one (batch, q-block) with filler
        # units woven after every k-tile; the segment's O-chains, normalize
        # and transpose work are RETURNED as units and run as fillers inside
        # the NEXT segment, so PE never stalls on the softmax barrier. ----
        def emit_seg(b2, qb, fillers=(), sch=SCH_EARLY, gates=None):
            fillers = list(fillers)
            gates = gates or {}
            qs = slice(b2 * SEQ + qb * TB, b2 * SEQ + (qb + 1) * TB)
            o_ps = [po.tile([P, NQB, HD + 1], F32, tag=f"o{h}",
                            name=f"o_{b2}_{qb}_{h}") for h in range(2)]
            e2s = []
            for kt in range(NKT):
                for u in gates.get(kt, ()):
                    u()
                kts = slice(b2 * SEQ + kt * P, b2 * SEQ + (kt + 1) * P)
                s2 = ps.tile([P, 2, TB], F32, tag="s", name=f"s2_{b2}_{qb}_{kt}")
                for h in range(2):
                    hb = HD * h
                    nc.tensor.matmul(s2[:, h, :],
                                     lhsT=kT[hb:hb + HD, kts],
                                     rhs=qT[hb:hb + HD, qs],
                                     start=True, stop=True)
                e2 = ep.tile([P, 2, TB], BF16, tag="e",
                             name=f"e2_{b2}_{qb}_{kt}")
                if kt in sch:
                    nc.vector.tensor_scalar(e2.bitcast(I16)[:, :, :], s2[:],
                                            A16, B16, OP.mult, OP.add)
                else:
                    nc.scalar.activation(e2[:], s2[:], AF.Exp,
                                         bias=0.0, scale=1.0)
                e2s.append(e2)
                for _ in range(2):
                    if fillers:
                        fillers.pop(0)()
            for f in fillers:
                f()

            def u_chain(qx, k0):
                # one accumulation group open per PSUM bank at a time; the
                # 4-ktile sub-chains stay in queue order so each (h, qx)
                # group runs without same-bank interleaving
                for kt in range(k0, k0 + 4):
                    for h in range(2):
                        nc.tensor.matmul(
                            o_ps[h][:, qx, :],
                            lhsT=e2s[kt][:, h, qx * P:(qx + 1) * P],
                            rhs=vtok[:, b2 * 2 + h, kt, :],
                            start=(kt == 0), stop=(kt == NKT - 1))

            oc = ocp.tile([P, NQB, 2, HD], BF16, tag="oc",
                          name=f"oc_{b2}_{qb}")

            def u_norm(h):
                # normalize by the ones-column denom (per-partition scalar)
                rc = rcp.tile([P, NQB], F32, tag="rc",
                              name=f"rc_{b2}_{qb}_{h}")
                nc.vector.reciprocal(rc[:], o_ps[h][:, :, HD:HD + 1])
                for qx in range(NQB):
                    nc.vector.tensor_scalar(oc[:, qx, h, :],
                                            o_ps[h][:, qx, 0:HD],
                                            rc[:, qx:qx + 1], None,
                                            OP.mult)

            def u_tr(qx):
                tr = pm.tile([P, P], BF16, tag="m",
                             name=f"tr_{b2}_{qb}_{qx}")
                nc.tensor.transpose(tr[:], oc[:, qx, :, :], ident[:, :])
                nc.vector.tensor_copy(
                    OT2[:, b2, qb * TB + qx * P: qb * TB + (qx + 1) * P],
                    tr[:])

            return ([lambda qx=qx, k0=k0: u_chain(qx, k0)
                     for qx in range(NQB) for k0 in range(0, NKT, 4)]
                    + [lambda h=h: u_norm(h) for h in range(2)]
                    + [lambda qx=qx: u_tr(qx) for qx in range(NQB)])

        # ---- interleaved, segment-pipelined emission ----
        def weave(a, b):
            out = []
            ia = ib = 0
            while ia < len(a) or ib < len(b):
                if ia < len(a):
                    out.append(a[ia]); ia += 1
                if ib < len(b):
                    out.append(b[ib]); ib += 1
            return out

        for tb in range(4):
            for u in phase1_units(tb, pfx=True):
                u()
        segs = [(0, qb) for qb in range(NQB)] + [(1, qb) for qb in range(NQB)]
        proj_units = (
            [lambda b2=b2, qt=qt, hf=hf: emit_proj_half(b2, qt, hf)
             for b2 in range(B) for qt in range(NQT) for hf in range(2)])
        # proj(b2, qt-group g) is eligible once tail(seg 4*b2+g) ran; projs
        # stay strictly after the tail units in each filler list
        per_seg_proj = {4: 10, 5: 16, 6: 16, 7: 12}
        tail = []
        for i, (b2, qb) in enumerate(segs):
            if i < 4:
                fillers = weave(tail, phase1_units(4 + i))
            else:
                n = min(per_seg_proj.get(i, 0), len(proj_units))
                fillers = tail + proj_units[:n]
                del proj_units[:n]
            tail = emit_seg(b2, qb, fillers,
                            sch=SCH_EARLY if i < 4 else SCH_LATE)
        for f in tail + proj_units:
            f()

        if os.environ.get("KV2_DEBUG2"):
            dts = nc.dram_tensor("dbg_tsb", [P, 2, TB], BF16,
                                 kind="ExternalOutput")
            dcq = nc.dram_tensor("dbg_coefq", [P, 2, TB], BF16,
                                 kind="ExternalOutput")
            dbg_c = const.tile([P, 2, TB], BF16, name="dbg_c")
            nc.vector.tensor_copy(dbg_c[:], dbg_tiles[1][:])
            nc.sync.dma_start(dts[:, :, :], dbg_tiles[0][:])
            nc.sync.dma_start(dcq[:, :, :], dbg_c[:])
        if os.environ.get("KV2_DEBUG"):
            dq = nc.dram_tensor("dbg_q", [P, TOKS], BF16, kind="ExternalOutput")
            dk = nc.dram_tensor("dbg_k", [P, TOKS], BF16, kind="ExternalOutput")
            dv = nc.dram_tensor("dbg_v", [P, TOKS], BF16, kind="ExternalOutput")
            dvt = nc.dram_tensor("dbg_vtok", [P, B * 2, NKT, HD + 1], BF16,
                                 kind="ExternalOutput")
            dot = nc.dram_tensor("dbg_ot2", [P, B, SEQ], BF16,
                                 kind="ExternalOutput")
            nc.sync.dma_start(dq[:, :], qT[:])
            nc.sync.dma_start(dk[:, :], kT[:])
            nc.sync.dma_start(dv[:, :], vT[:])
            nc.sync.dma_start(dvt[:, :, :, :], vtok[:])
            nc.sync.dma_start(dot[:, :, :], OT2[:])


_NC_CACHE = None


def build_nc():
    global _NC_CACHE
    if _NC_CACHE is None:
        nc = bacc.Bacc("TRN2", target_bir_lowering=False, debug=False)
        with tile.TileContext(nc) as tc:
            _emit(tc)
        nc.compile()
        _NC_CACHE = nc
    return _NC_CACHE


def make_in_maps(x, w_qkv, b_qkv, g_q, be_q, g_k, be_k, w_proj):
    x2 = np.ascontiguousarray(np.asarray(x, np.float32).reshape(TOKS, C))
    w_qkv = np.asarray(w_qkv, np.float32)
    b_qkv = np.asarray(b_qkv, np.float32)
    g_q = np.asarray(g_q, np.float32)
    be_q = np.asarray(be_q, np.float32)
    g_k = np.asarray(g_k, np.float32)
    be_k = np.asarray(be_k, np.float32)
    w_proj = np.asarray(w_proj, np.float32)

    # x fp8 residual pair, laid out [tb, p, ko, t]
    xh_f = x2.astype(NPF8)
    xl_f = (x2 - xh_f.astype(np.float32)).astype(NPF8)

    def to_xT(a):
        return np.ascontiguousarray(
            a.T.reshape(KO, P, NTB, TB).transpose(2, 1, 0, 3))

    xh_h = to_xT(xh_f)
    xl_h = to_xT(xl_f)

    sel_h = np.zeros((P, 2), NPBF)
    sel_h[0:HD, 0] = np.float32(1.0 / HD)
    sel_h[HD:P, 1] = np.float32(1.0 / HD)
    grows_h = np.zeros((66, 2, P), NPBF)
    for r0, gv in ((0, g_q / 8.0), (64, g_k)):
        grows_h[r0, 0, 0:HD] = gv
        grows_h[r0 + 1, 0, HD:P] = gv
    gbe_h = np.stack([np.tile(g_q, 2) / 8.0, np.tile(be_q, 2) / 8.0,
                      np.tile(g_k, 2), np.tile(be_k, 2),
                      np.full(P, EPS, np.float32),
                      np.ones(P, np.float32)], axis=1)
    gbe_h = np.ascontiguousarray(gbe_h.astype(np.float32))
    idd_h = np.eye(P, dtype=NPBF)

    in_maps = []
    for c in range(NCORES):
        cs = slice(P * c, P * (c + 1))
        wcat = np.concatenate(
            [w_qkv[:, 0:C][:, cs], w_qkv[:, C:2 * C][:, cs],
             w_qkv[:, 2 * C:3 * C][:, cs]], axis=1) * np.float32(WS)
        wh_f = wcat.astype(NPF8)
        wl_f = (wcat - wh_f.astype(np.float32)).astype(NPF8)
        # [p, hl, ko, col]
        w_h = np.ascontiguousarray(np.stack(
            [wh_f.reshape(KO, P, 384).transpose(1, 0, 2),
             wl_f.reshape(KO, P, 384).transpose(1, 0, 2)], axis=1))
        bcat = np.concatenate(
            [b_qkv[0:C][cs], b_qkv[C:2 * C][cs], b_qkv[2 * C:3 * C][cs]])
        b_h = np.ascontiguousarray(bcat.reshape(3, P).T.astype(np.float32))
        wp_h = np.ascontiguousarray(w_proj[cs, :].astype(NPBF))
        in_maps.append({
            "xh": xh_h, "xl": xl_h, "wq": w_h, "bq3": b_h,
            "selm": sel_h, "grows": grows_h, "gbe": gbe_h,
            "wp": wp_h, "idd": idd_h,
        })
    return in_maps


def kernel(x, w_qkv, b_qkv, g_q, be_q, g_k, be_k, w_proj, b_proj, **run_kwargs):
    in_maps = make_in_maps(x, w_qkv, b_qkv, g_q, be_q, g_k, be_k, w_proj)
    nc = build_nc()
    res = run_bass_kernel_spmd(nc, in_maps, list(range(NCORES)), **run_kwargs)
    acc = np.zeros((TOKS, C), np.float32)
    for r in res.results:
        yv = r["y"]
        if yv.dtype != NPBF:
            yv = yv.view(NPBF)
        acc += yv.reshape(TOKS, C).astype(np.float32)
    out = acc + np.asarray(b_proj, np.float32)
    out = out.astype(np.float32).reshape(B, SEQ, C)
    kernel.last_result = res
    return out
